# revision 7
# baseline (speedup 1.0000x reference)
"""Depthwise cross-correlation (per-sample dynamic kernel) on 8 Trainium2 cores.

reference: out[b,i,j,c] = sum_{di,dj} search[b,i+di,j+dj,c] * template[b,di,dj,c]
  search [64,31,31,256] f32, template [64,7,7,256] f32 -> out [64,25,25,256] f32

Strategy (pure data parallel, 8 samples/core = 16 independent
(sample, channel-half) units, no collectives), all fp16 on device:
- The 49 taps are split across all four compute engines and balanced
  against the cost model:
  * PE (N_PE taps): diag(t_k) @ shift_k(S) accumulated in PSUM, fp16
    weights/data (1 cycle/row, no fp32r even-count constraint -> 25-wide
    windows). Two PSUM banks per unit (13+12 output rows).
  * ACT (N_ACT taps): activation-Copy with per-partition fp32 scale into
    a shared product buffer.
  * DVE (N_DVE taps): tensor_scalar muls (4x perf mode with packed fp16
    SBUF operands) into the product buffer, then a binary-tree reduction
    of all products with wide tensor_tensor adds (2x mode).
  * Pool/GPSIMD (N_POOL taps): fused scalar_tensor_tensor mul-add chain
    into the last product slot.
- DVE folds PSUM + product-tree root into the fp16 output tile via
  scalar_tensor_tensor; host upcasts fp16 -> fp32.
- fp16 everywhere cuts DMA from 45 MB/core (fp32 + fp32 diags) to
  ~19 MB/core; rel err ~1e-3 << 2e-2 gate.
- A post-pass splits multi-wait instructions (walrus allows one sync-wait
  per instruction) into single-wait NoOp carriers.
"""
import sys

sys.path.insert(0, "/opt/trn_rl_repo")

import numpy as np
import concourse.bass as bass
import concourse.mybir as mybir
import concourse.tile as tile
from concourse.bass_utils import run_bass_kernel_spmd

B = 64
X, K, OUT = 31, 7, 25
KK = K * K                   # 49 taps
CH = 256
C = 128                      # channels per half (partition dim)
N_CORES = 8
BPC = B // N_CORES           # samples per core
UNITS = BPC * 2              # (sample, half) units per core

# tap split across engines (sum must be 49)
N_PE = 30
N_ACT = 10
N_DVE = 3
N_POOL = 6
assert N_PE + N_ACT + N_DVE + N_POOL == KK
NPROD = N_ACT + N_DVE + N_POOL   # product slots

SLEN = X * X                 # 961
T16 = SLEN                   # fp16 template values (49, padded to 65)
T32 = SLEN + 65              # fp32 template values as 98 fp16 slots (pad 128)
DGOFF = T32 + 128            # diag tiles for the PE taps
SECT = DGOFF + N_PE * 128    # per-unit section (fp16 elems, even)
assert T32 % 2 == 0 and SECT % 2 == 0
R0, R1 = 13, 12              # output row split (325 / 300 <= 512 psum bank)

PE_TAPS = list(range(0, N_PE))
ACT_TAPS = list(range(N_PE, N_PE + N_ACT))
DVE_TAPS = list(range(N_PE + N_ACT, N_PE + N_ACT + N_DVE))
POOL_TAPS = list(range(N_PE + N_ACT + N_DVE, KK))

_CACHE = {}


def _split_excess_waits(nc):
    """Walrus codegen allows a single sync-wait command per instruction.
    Move extra waits onto inserted same-engine NoOps; firing a monotone
    wait earlier on the same queue is always safe."""
    for fn in nc.m.functions:
        for bb in fn.blocks:
            out = []
            for inst in bb.instructions:
                si = inst.sync_info
                if si is not None and len(si.on_wait) > 1:
                    waits = list(si.on_wait)
                    for w in waits[:-1]:
                        nop = mybir.InstNoOp(
                            name=nc.get_next_instruction_name(), ins=[], outs=[])
                        nop.engine = inst.engine
                        nop.sync_info = mybir.SyncInfo(on_wait=[w], on_update=[])
                        out.append(nop)
                    si.on_wait = [waits[-1]]
                out.append(inst)
            bb.instructions = out


def _build_nc(reps=1):
    nc = bass.Bass("TRN2", debug=False)
    f16 = mybir.dt.float16
    f32 = mybir.dt.float32
    mult = mybir.AluOpType.mult
    add = mybir.AluOpType.add
    b_in = nc.dram_tensor("blob", [UNITS, C, SECT], f16,
                          kind="ExternalInput").ap()
    o_out = nc.dram_tensor("o", [UNITS, C, OUT, OUT], f16,
                           kind="ExternalOutput").ap()

    with tile.TileContext(nc) as tc:
        with tc.tile_pool(name="sb", bufs=3) as sb, \
             tc.tile_pool(name="work", bufs=2) as work, \
             tc.tile_pool(name="ps", bufs=3, space="PSUM") as ps:
            for _ in range(reps):
                for u in range(UNITS):
                    blob = sb.tile([C, SECT], f16, tag="blob")
                    nc.sync.dma_start(out=blob[:], in_=b_in[u])

                    def rows(off, nr):
                        return blob[:, off: off + X * nr].rearrange(
                            "c (r j) -> c r j", j=X)[:, :, 0:OUT]

                    def t32col(k):
                        return blob[:, T32 + 2 * k: T32 + 2 * k + 2].bitcast(f32)

                    P = work.tile([C, NPROD, OUT, OUT], f16, tag="prod")
                    out_sb = work.tile([C, OUT, OUT], f16, tag="out")

                    pa = ps.tile([C, R0, OUT], f32, tag="pa")
                    pb = ps.tile([C, R1, OUT], f32, tag="pb")

                    # ACT taps: per-partition scale multiplies
                    for i, k in enumerate(ACT_TAPS):
                        di, dj = divmod(k, K)
                        nc.scalar.mul(P[:, i], rows(di * X + dj, OUT),
                                      t32col(k))
                    # DVE taps: tensor_scalar muls (4x mode)
                    for i, k in enumerate(DVE_TAPS):
                        di, dj = divmod(k, K)
                        nc.vector.tensor_scalar_mul(
                            P[:, N_ACT + i], rows(di * X + dj, OUT),
                            t32col(k))
                    # Pool taps: tensor_scalar muls
                    for i, k in enumerate(POOL_TAPS):
                        di, dj = divmod(k, K)
                        nc.gpsimd.tensor_scalar_mul(
                            P[:, N_ACT + N_DVE + i],
                            rows(di * X + dj, OUT), t32col(k))

                    # PE taps: diag matmuls accumulating in PSUM
                    dsec = blob[:, DGOFF:].rearrange("c (k m) -> c k m",
                                                     k=N_PE)
                    for j, k in enumerate(PE_TAPS):
                        di, dj = divmod(k, K)
                        for (pt, rb, nr) in ((pa, 0, R0), (pb, R0, R1)):
                            nc.tensor.matmul(
                                pt[:, :, :], dsec[:, j, :],
                                rows((rb + di) * X + dj, nr),
                                start=(j == 0), stop=(j == N_PE - 1),
                                skip_group_check=True)

                    # DVE: tree-reduce the product slots
                    m = NPROD
                    while m > 1:
                        if m % 2:
                            nc.vector.tensor_add(out=P[:, m - 2],
                                                 in0=P[:, m - 2],
                                                 in1=P[:, m - 1])
                            m -= 1
                        h = m // 2
                        nc.vector.tensor_add(out=P[:, 0:h], in0=P[:, 0:h],
                                             in1=P[:, h:m])
                        m = h
                    # folds: out = psum + tree root
                    nc.vector.scalar_tensor_tensor(
                        out_sb[:, 0:R0], pa[:, :, :], 1.0, P[:, 0, 0:R0],
                        op0=mult, op1=add)
                    nc.vector.scalar_tensor_tensor(
                        out_sb[:, R0:OUT], pb[:, :, :], 1.0, P[:, 0, R0:OUT],
                        op0=mult, op1=add)
                    nc.sync.dma_start(out=o_out[u], in_=out_sb[:])
    _split_excess_waits(nc)
    return nc


def _marshal(search, template):
    """-> blob [N_CORES, UNITS, C, SECT] float16."""
    search = np.ascontiguousarray(search, dtype=np.float32)
    template = np.ascontiguousarray(template, dtype=np.float32)
    s_cm = search.reshape(B, SLEN, 2, C).transpose(0, 2, 3, 1)      # [B,2,C,961]
    t_cm32 = np.ascontiguousarray(
        template.reshape(B, KK, 2, C).transpose(0, 2, 3, 1))        # [B,2,C,49]
    t_cm = t_cm32.astype(np.float16)
    blob = np.zeros((B, 2, C, SECT), np.float16)
    blob[..., :SLEN] = s_cm.astype(np.float16)
    blob[..., T16:T16 + KK] = t_cm
    blob[..., T32:T32 + 2 * KK] = t_cm32.view(np.float16)
    d = blob[..., DGOFF:].reshape(B, 2, C, N_PE, 128)
    c = np.arange(C)
    d[:, :, c, :, c] = t_cm[:, :, :, PE_TAPS].transpose(2, 0, 1, 3)
    return np.ascontiguousarray(blob.reshape(N_CORES, UNITS, C, SECT))


def _unmarshal(results):
    o = np.stack([results[core]["o"] for core in range(N_CORES)])
    # [cores, UNITS, C, OUT, OUT] -> [B, 2, C, OUT, OUT]
    o = o.reshape(B, 2, C, OUT, OUT).transpose(0, 3, 4, 1, 2)
    return np.ascontiguousarray(o.reshape(B, OUT, OUT, CH), dtype=np.float32)


def kernel(search, template):
    if "nc" not in _CACHE:
        _CACHE["nc"] = _build_nc()
    nc = _CACHE["nc"]
    blob = _marshal(search, template)
    in_maps = [{"blob": blob[core]} for core in range(N_CORES)]
    res = run_bass_kernel_spmd(nc, in_maps, core_ids=list(range(N_CORES)))
    return _unmarshal(res.results)


# revision 21
# speedup vs baseline: 1.1058x; 1.1058x over previous
"""Depthwise cross-correlation (per-sample dynamic kernel) on 8 Trainium2 cores.

reference: out[b,i,j,c] = sum_{di,dj} search[b,i+di,j+dj,c] * template[b,di,dj,c]
  search [64,31,31,256] f32, template [64,7,7,256] f32 -> out [64,25,25,256] f32

Strategy (pure data parallel, 8 samples/core = 16 independent
(sample, channel-half) units, no collectives), all fp16 on device:
- The 49 taps are split across all four compute engines and balanced
  against the cost model:
  * PE (N_PE taps): diag(t_k) @ shift_k(S) accumulated in PSUM, fp16
    weights/data (1 cycle/row, no fp32r even-count constraint -> 25-wide
    windows). Two PSUM banks per unit (13+12 output rows).
  * ACT (N_ACT taps): activation-Copy with per-partition fp32 scale into
    a shared product buffer.
  * DVE (N_DVE taps): tensor_scalar muls (4x perf mode with packed fp16
    SBUF operands) into the product buffer, then a binary-tree reduction
    of all products with wide tensor_tensor adds (2x mode).
  * Pool/GPSIMD (N_POOL taps): fused scalar_tensor_tensor mul-add chain
    into the last product slot.
- DVE folds PSUM + product-tree root into the fp16 output tile via
  scalar_tensor_tensor; host upcasts fp16 -> fp32.
- fp16 everywhere cuts DMA from 45 MB/core (fp32 + fp32 diags) to
  ~19 MB/core; rel err ~1e-3 << 2e-2 gate.
- A post-pass splits multi-wait instructions (walrus allows one sync-wait
  per instruction) into single-wait NoOp carriers.
"""
import sys

sys.path.insert(0, "/opt/trn_rl_repo")

import numpy as np
import concourse.bass as bass
import concourse.mybir as mybir
import concourse.tile as tile
from concourse.bass_utils import run_bass_kernel_spmd

B = 64
X, K, OUT = 31, 7, 25
KK = K * K                   # 49 taps
CH = 256
C = 128                      # channels per half (partition dim)
N_CORES = 8
BPC = B // N_CORES           # samples per core
UNITS = BPC * 2              # (sample, half) units per core

# tap split across engines (sum must be 49)
N_PE = 31
N_ACT = 11
N_DVE = 7
N_POOL = 0
assert N_PE + N_ACT + N_DVE + N_POOL == KK
NPROD = N_ACT + N_DVE + N_POOL   # product slots

SLEN = X * X                 # 961
SPAD = 992                   # search section padded to 32 rows (row 31 = 0)
T16 = SPAD                   # fp16 template values (49, padded to 66)
T32 = SPAD + 66              # fp32 template values as 98 fp16 slots (pad 128)
DGOFF = T32 + 128            # diag tiles for the PE taps
SECT = DGOFF + N_PE * 128    # per-unit section (fp16 elems, even)
assert T32 % 2 == 0 and SECT % 2 == 0
R0, R1 = 13, 12              # output row split (325 / 300 <= 512 psum bank)

BUFS_SB = 3
BUFS_WORK = 2
BUFS_PS = 3
N_DMA_ADDS = 0               # tail product-slot pairs folded by accum-DMA

PE_TAPS = list(range(0, N_PE))
ACT_TAPS = list(range(N_PE, N_PE + N_ACT))
DVE_TAPS = list(range(N_PE + N_ACT, N_PE + N_ACT + N_DVE))
POOL_TAPS = list(range(N_PE + N_ACT + N_DVE, KK))

_CACHE = {}


def _split_excess_waits(nc):
    """Walrus codegen allows a single sync-wait command per instruction.
    Move extra waits onto inserted same-engine NoOps; firing a monotone
    wait earlier on the same queue is always safe."""
    for fn in nc.m.functions:
        for bb in fn.blocks:
            out = []
            for inst in bb.instructions:
                si = inst.sync_info
                if si is not None and len(si.on_wait) > 1:
                    waits = list(si.on_wait)
                    for w in waits[:-1]:
                        nop = mybir.InstNoOp(
                            name=nc.get_next_instruction_name(), ins=[], outs=[])
                        nop.engine = inst.engine
                        nop.sync_info = mybir.SyncInfo(on_wait=[w], on_update=[])
                        out.append(nop)
                    si.on_wait = [waits[-1]]
                out.append(inst)
            bb.instructions = out


def _build_nc(reps=1):
    nc = bass.Bass("TRN2", debug=False)
    f16 = mybir.dt.float16
    f32 = mybir.dt.float32
    mult = mybir.AluOpType.mult
    add = mybir.AluOpType.add
    b_in = nc.dram_tensor("blob", [UNITS, C, SECT], f16,
                          kind="ExternalInput").ap()
    o_out = nc.dram_tensor("o", [UNITS, C, OUT, OUT], f16,
                           kind="ExternalOutput").ap()

    with tile.TileContext(nc) as tc:
        with tc.tile_pool(name="sb", bufs=BUFS_SB) as sb, \
             tc.tile_pool(name="work", bufs=BUFS_WORK) as work, \
             tc.tile_pool(name="ps", bufs=BUFS_PS, space="PSUM") as ps:
            for _ in range(reps):
                for u in range(UNITS):
                    blob = sb.tile([C, SECT], f16, tag="blob")
                    nc.sync.dma_start(out=blob[:], in_=b_in[u])

                    def rows(off, nr):
                        return blob[:, off: off + X * nr].rearrange(
                            "c (r j) -> c r j", j=X)[:, :, 0:OUT]

                    def t32col(k):
                        return blob[:, T32 + 2 * k: T32 + 2 * k + 2].bitcast(f32)

                    P = work.tile([C, NPROD, OUT, OUT], f16, tag="prod")
                    out_sb = work.tile([C, 2, R0 * OUT], f16, tag="out")

                    # one PSUM tile spanning 2 banks; the matmul output AP
                    # covers both banks (rows 0-12 in bank 0, 13-25 in bank 1
                    # at elem offset 512; out row 25 is computed-but-unused).
                    pp = ps.tile([C, 2, 512], f32, tag="pp")
                    pview = pp[:, :, 0:R0 * OUT]

                    # ACT taps: per-partition scale multiplies
                    for i, k in enumerate(ACT_TAPS):
                        di, dj = divmod(k, K)
                        nc.scalar.mul(P[:, i], rows(di * X + dj, OUT),
                                      t32col(k))
                    # DVE taps: tensor_scalar muls (4x mode)
                    for i, k in enumerate(DVE_TAPS):
                        di, dj = divmod(k, K)
                        nc.vector.tensor_scalar_mul(
                            P[:, N_ACT + i], rows(di * X + dj, OUT),
                            t32col(k))
                    # Pool taps: tensor_scalar muls
                    for i, k in enumerate(POOL_TAPS):
                        di, dj = divmod(k, K)
                        nc.gpsimd.tensor_scalar_mul(
                            P[:, N_ACT + N_DVE + i],
                            rows(di * X + dj, OUT), t32col(k))

                    # PE taps: diag matmuls accumulating in PSUM, one matmul
                    # per tap covering both banks via a bank-strided rhs view
                    # (bank stride = 13 rows = 403 elems in the search grid).
                    dsec = blob[:, DGOFF:].rearrange("c (k m) -> c k m",
                                                     k=N_PE)
                    for j, k in enumerate(PE_TAPS):
                        di, dj = divmod(k, K)
                        for b in range(2):
                            nc.tensor.matmul(
                                pp[:, b, 0:R0 * OUT], dsec[:, j, :],
                                rows((b * R0 + di) * X + dj, R0),
                                start=(j == 0), stop=(j == N_PE - 1),
                                skip_group_check=True)

                    # accum-DMAs (pool-dispatched, add on DMA engines) fold
                    # the last N_DMA_ADDS slot pairs before the DVE tree
                    d = N_DMA_ADDS
                    for i in range(d):
                        nc.gpsimd.dma_start(
                            out=P[:, NPROD - 2 * d + i],
                            in_=P[:, NPROD - d + i],
                            accum_op=add)
                    # DVE: tree-reduce the product slots
                    m = NPROD - d
                    while m > 1:
                        if m % 2:
                            nc.vector.tensor_add(out=P[:, m - 2],
                                                 in0=P[:, m - 2],
                                                 in1=P[:, m - 1])
                            m -= 1
                        h = m // 2
                        nc.vector.tensor_add(out=P[:, 0:h], in0=P[:, 0:h],
                                             in1=P[:, h:m])
                        m = h
                    # single fold: out = psum + tree root over both banks
                    # (row 25 of the bank view is garbage, skipped by the
                    # output DMAs; its tree-root view reads into slot 1).
                    Pf = P.rearrange("c n r j -> c (n r j)")
                    root = Pf[:, 0:2 * R0 * OUT].rearrange(
                        "c (b q) -> c b q", b=2)
                    nc.vector.scalar_tensor_tensor(
                        out_sb[:, :, :], pview, 1.0, root,
                        op0=mult, op1=add)
                    nc.sync.dma_start(
                        out=o_out[u, :, 0:R0],
                        in_=out_sb[:, 0].rearrange("c (r j) -> c r j", j=OUT))
                    nc.sync.dma_start(
                        out=o_out[u, :, R0:OUT],
                        in_=out_sb[:, 1, 0:R1 * OUT].rearrange(
                            "c (r j) -> c r j", j=OUT))
    _split_excess_waits(nc)
    return nc


def _marshal(search, template):
    """-> blob [N_CORES, UNITS, C, SECT] float16."""
    search = np.ascontiguousarray(search, dtype=np.float32)
    template = np.ascontiguousarray(template, dtype=np.float32)
    s_cm = search.reshape(B, SLEN, 2, C).transpose(0, 2, 3, 1)      # [B,2,C,961]
    t_cm32 = np.ascontiguousarray(
        template.reshape(B, KK, 2, C).transpose(0, 2, 3, 1))        # [B,2,C,49]
    t_cm = t_cm32.astype(np.float16)
    blob = np.zeros((B, 2, C, SECT), np.float16)
    blob[..., :SLEN] = s_cm.astype(np.float16)
    blob[..., T16:T16 + KK] = t_cm
    blob[..., T32:T32 + 2 * KK] = t_cm32.view(np.float16)
    d = blob[..., DGOFF:].reshape(B, 2, C, N_PE, 128)
    c = np.arange(C)
    d[:, :, c, :, c] = t_cm[:, :, :, PE_TAPS].transpose(2, 0, 1, 3)
    return np.ascontiguousarray(blob.reshape(N_CORES, UNITS, C, SECT))


def _unmarshal(results):
    o = np.stack([results[core]["o"] for core in range(N_CORES)])
    # [cores, UNITS, C, OUT, OUT] -> [B, 2, C, OUT, OUT]
    o = o.reshape(B, 2, C, OUT, OUT).transpose(0, 3, 4, 1, 2)
    return np.ascontiguousarray(o.reshape(B, OUT, OUT, CH), dtype=np.float32)


def kernel(search, template):
    if "nc" not in _CACHE:
        _CACHE["nc"] = _build_nc()
    nc = _CACHE["nc"]
    blob = _marshal(search, template)
    in_maps = [{"blob": blob[core]} for core in range(N_CORES)]
    res = run_bass_kernel_spmd(nc, in_maps, core_ids=list(range(N_CORES)))
    return _unmarshal(res.results)


# revision 25
# speedup vs baseline: 1.1075x; 1.0016x over previous
"""Depthwise cross-correlation (per-sample dynamic kernel) on 8 Trainium2 cores.

reference: out[b,i,j,c] = sum_{di,dj} search[b,i+di,j+dj,c] * template[b,di,dj,c]
  search [64,31,31,256] f32, template [64,7,7,256] f32 -> out [64,25,25,256] f32

Strategy (pure data parallel, 8 samples/core = 16 independent
(sample, channel-half) units, no collectives), all fp16 on device:
- The 49 taps are split across three engines (GPSIMD measured ~8x slower
  than its cost model on strided tensor_scalar -> unused):
  * PE (31 taps): diag(t_k) @ shift_k(S) accumulated in PSUM, fp16
    weights/data (1 cycle/row; 25-wide windows, no fp32r even-count
    constraint). One [C,2,512] PSUM tile: 13 output rows per bank
    (bank-1 row 25 is computed from a zero-padded search row 31 and
    dropped on the host), so each tap is two 325-row matmuls sharing
    one accumulation group and the fold is a single 650-wide op.
  * ACT (11 taps): activation-Copy with per-partition fp32 scale into
    a shared product buffer.
  * DVE (7 taps): tensor_scalar muls (4x perf mode with packed fp16
    SBUF operands) into the product buffer, then a binary-tree reduction
    of all 18 products with wide tensor_tensor adds (2x mode).
- One DVE scalar_tensor_tensor folds PSUM + tree root into the fp16
  output tile; one DMA per unit writes [C,2,325]; host drops the
  garbage row and upcasts fp16 -> fp32.
- fp16 everywhere cuts DMA from 45 MB/core (fp32 + fp32 diags) to
  ~15 MB/core; rel err ~6e-4 << 2e-2 gate.
- Engine-written PSUM is NOT accumulated by start=False matmuls (tested:
  the seed contribution is lost), so everything non-PE goes through the
  SBUF product tree instead. DMA accum_op=add (gpsimd SWDGE) compiles
  but returns wrong sums on this runtime -> unused.
- A post-pass splits multi-wait instructions (walrus allows one sync-wait
  per instruction) into single-wait NoOp carriers.
"""
import sys

sys.path.insert(0, "/opt/trn_rl_repo")

import numpy as np
import concourse.bass as bass
import concourse.mybir as mybir
import concourse.tile as tile
from concourse.bass_utils import run_bass_kernel_spmd

B = 64
X, K, OUT = 31, 7, 25
KK = K * K                   # 49 taps
CH = 256
C = 128                      # channels per half (partition dim)
N_CORES = 8
BPC = B // N_CORES           # samples per core
UNITS = BPC * 2              # (sample, half) units per core

# tap split across engines (sum must be 49)
N_PE = 31
N_ACT = 11
N_DVE = 7
N_POOL = 0
assert N_PE + N_ACT + N_DVE + N_POOL == KK
NPROD = N_ACT + N_DVE + N_POOL   # product slots

SLEN = X * X                 # 961
SPAD = 992                   # search section padded to 32 rows (row 31 = 0)
T16 = SPAD                   # fp16 template values (49, padded to 66)
T32 = SPAD + 66              # fp32 template values as 98 fp16 slots (pad 128)
DGOFF = T32 + 128            # diag tiles for the PE taps
SECT = DGOFF + N_PE * 128    # per-unit section (fp16 elems, even)
assert T32 % 2 == 0 and SECT % 2 == 0
R0, R1 = 13, 12              # output row split (325 / 300 <= 512 psum bank)

BUFS_SB = 3
BUFS_WORK = 2
BUFS_PS = 3
N_DMA_ADDS = 0               # tail product-slot pairs folded by accum-DMA

PE_TAPS = list(range(0, N_PE))
ACT_TAPS = list(range(N_PE, N_PE + N_ACT))
DVE_TAPS = list(range(N_PE + N_ACT, N_PE + N_ACT + N_DVE))
POOL_TAPS = list(range(N_PE + N_ACT + N_DVE, KK))

_CACHE = {}


def _split_excess_waits(nc):
    """Walrus codegen allows a single sync-wait command per instruction.
    Move extra waits onto inserted same-engine NoOps; firing a monotone
    wait earlier on the same queue is always safe."""
    for fn in nc.m.functions:
        for bb in fn.blocks:
            out = []
            for inst in bb.instructions:
                si = inst.sync_info
                if si is not None and len(si.on_wait) > 1:
                    waits = list(si.on_wait)
                    for w in waits[:-1]:
                        nop = mybir.InstNoOp(
                            name=nc.get_next_instruction_name(), ins=[], outs=[])
                        nop.engine = inst.engine
                        nop.sync_info = mybir.SyncInfo(on_wait=[w], on_update=[])
                        out.append(nop)
                    si.on_wait = [waits[-1]]
                out.append(inst)
            bb.instructions = out


def _build_nc(reps=1):
    nc = bass.Bass("TRN2", debug=False)
    f16 = mybir.dt.float16
    f32 = mybir.dt.float32
    mult = mybir.AluOpType.mult
    add = mybir.AluOpType.add
    b_in = nc.dram_tensor("blob", [UNITS, C, SECT], f16,
                          kind="ExternalInput").ap()
    # output as 2 banks x 13 rows; row 25 (bank 1 row 12) is garbage and
    # dropped on the host
    o_out = nc.dram_tensor("o", [UNITS, C, 2, R0 * OUT], f16,
                           kind="ExternalOutput").ap()

    with tile.TileContext(nc) as tc:
        with tc.tile_pool(name="sb", bufs=BUFS_SB) as sb, \
             tc.tile_pool(name="work", bufs=BUFS_WORK) as work, \
             tc.tile_pool(name="ps", bufs=BUFS_PS, space="PSUM") as ps:
            for _ in range(reps):
                for u in range(UNITS):
                    blob = sb.tile([C, SECT], f16, tag="blob")
                    nc.sync.dma_start(out=blob[:], in_=b_in[u])

                    def rows(off, nr):
                        return blob[:, off: off + X * nr].rearrange(
                            "c (r j) -> c r j", j=X)[:, :, 0:OUT]

                    def t32col(k):
                        return blob[:, T32 + 2 * k: T32 + 2 * k + 2].bitcast(f32)

                    P = work.tile([C, NPROD, OUT, OUT], f16, tag="prod")
                    out_sb = work.tile([C, 2, R0 * OUT], f16, tag="out")

                    # one PSUM tile spanning 2 banks; the matmul output AP
                    # covers both banks (rows 0-12 in bank 0, 13-25 in bank 1
                    # at elem offset 512; out row 25 is computed-but-unused).
                    pp = ps.tile([C, 2, 512], f32, tag="pp")
                    pview = pp[:, :, 0:R0 * OUT]

                    # ACT taps: per-partition scale multiplies
                    for i, k in enumerate(ACT_TAPS):
                        di, dj = divmod(k, K)
                        nc.scalar.mul(P[:, i], rows(di * X + dj, OUT),
                                      t32col(k))
                    # DVE taps: tensor_scalar muls (4x mode)
                    for i, k in enumerate(DVE_TAPS):
                        di, dj = divmod(k, K)
                        nc.vector.tensor_scalar_mul(
                            P[:, N_ACT + i], rows(di * X + dj, OUT),
                            t32col(k))
                    # Pool taps: tensor_scalar muls
                    for i, k in enumerate(POOL_TAPS):
                        di, dj = divmod(k, K)
                        nc.gpsimd.tensor_scalar_mul(
                            P[:, N_ACT + N_DVE + i],
                            rows(di * X + dj, OUT), t32col(k))

                    # PE taps: diag matmuls accumulating in PSUM, one matmul
                    # per tap covering both banks via a bank-strided rhs view
                    # (bank stride = 13 rows = 403 elems in the search grid).
                    dsec = blob[:, DGOFF:].rearrange("c (k m) -> c k m",
                                                     k=N_PE)
                    for j, k in enumerate(PE_TAPS):
                        di, dj = divmod(k, K)
                        for b in range(2):
                            nc.tensor.matmul(
                                pp[:, b, 0:R0 * OUT], dsec[:, j, :],
                                rows((b * R0 + di) * X + dj, R0),
                                start=(j == 0), stop=(j == N_PE - 1),
                                skip_group_check=True)

                    # accum-DMAs (pool-dispatched, add on DMA engines) fold
                    # the last N_DMA_ADDS slot pairs before the DVE tree
                    d = N_DMA_ADDS
                    for i in range(d):
                        nc.gpsimd.dma_start(
                            out=P[:, NPROD - 2 * d + i],
                            in_=P[:, NPROD - d + i],
                            accum_op=add)
                    # DVE: tree-reduce the product slots
                    m = NPROD - d
                    while m > 1:
                        if m % 2:
                            nc.vector.tensor_add(out=P[:, m - 2],
                                                 in0=P[:, m - 2],
                                                 in1=P[:, m - 1])
                            m -= 1
                        h = m // 2
                        nc.vector.tensor_add(out=P[:, 0:h], in0=P[:, 0:h],
                                             in1=P[:, h:m])
                        m = h
                    # single fold: out = psum + tree root over both banks
                    # (row 25 of the bank view is garbage, skipped by the
                    # output DMAs; its tree-root view reads into slot 1).
                    Pf = P.rearrange("c n r j -> c (n r j)")
                    root = Pf[:, 0:2 * R0 * OUT].rearrange(
                        "c (b q) -> c b q", b=2)
                    nc.vector.scalar_tensor_tensor(
                        out_sb[:, :, :], pview, 1.0, root,
                        op0=mult, op1=add)
                    nc.sync.dma_start(out=o_out[u], in_=out_sb[:])
    _split_excess_waits(nc)
    return nc


def _marshal(search, template):
    """-> blob [N_CORES, UNITS, C, SECT] float16."""
    search = np.ascontiguousarray(search, dtype=np.float32)
    template = np.ascontiguousarray(template, dtype=np.float32)
    s_cm = search.reshape(B, SLEN, 2, C).transpose(0, 2, 3, 1)      # [B,2,C,961]
    t_cm32 = np.ascontiguousarray(
        template.reshape(B, KK, 2, C).transpose(0, 2, 3, 1))        # [B,2,C,49]
    t_cm = t_cm32.astype(np.float16)
    blob = np.zeros((B, 2, C, SECT), np.float16)
    blob[..., :SLEN] = s_cm.astype(np.float16)
    blob[..., T16:T16 + KK] = t_cm
    blob[..., T32:T32 + 2 * KK] = t_cm32.view(np.float16)
    d = blob[..., DGOFF:].reshape(B, 2, C, N_PE, 128)
    c = np.arange(C)
    d[:, :, c, :, c] = t_cm[:, :, :, PE_TAPS].transpose(2, 0, 1, 3)
    return np.ascontiguousarray(blob.reshape(N_CORES, UNITS, C, SECT))


def _unmarshal(results):
    o = np.stack([results[core]["o"] for core in range(N_CORES)])
    # [cores, UNITS, C, 2, 325] -> rows 0..25 (last garbage) -> [B,2,C,25,25]
    o = o.reshape(B, 2, C, 2 * R0 * OUT)[:, :, :, :OUT * OUT]
    o = o.reshape(B, 2, C, OUT, OUT).transpose(0, 3, 4, 1, 2)
    return np.ascontiguousarray(o.reshape(B, OUT, OUT, CH), dtype=np.float32)


def kernel(search, template):
    if "nc" not in _CACHE:
        _CACHE["nc"] = _build_nc()
    nc = _CACHE["nc"]
    blob = _marshal(search, template)
    in_maps = [{"blob": blob[core]} for core in range(N_CORES)]
    res = run_bass_kernel_spmd(nc, in_maps, core_ids=list(range(N_CORES)))
    return _unmarshal(res.results)
